# revision 9
# baseline (speedup 1.0000x reference)
"""LIF spike (leaky integrate-and-fire) forward kernel for Trainium2.

Recurrence over the innermost time axis T=8 of x[64,128,32,32,8] (fp32):
    u_t = TAU * u_{t-1} * (1 - o_{t-1}) + x_t
    o_t = (u_t > VTH)
Data-parallel over the batch dim: 8 NeuronCores x 8 batches each.

Per-core layout: the 32 MiB shard is viewed as [2048 rows, 4096 cols]
(each row = one (b, c, h-half) slab, cols = spatial*T contiguous). Tiles
of [128, 4096] stream HBM->SBUF; inside SBUF the recurrence walks the
stride-8 time slices in place (the x tile doubles as the membrane-state
buffer), spikes are produced by ScalarE (Sign+Relu) while VectorE does the
reset (copy_predicated) and leak+integrate (scalar_tensor_tensor), so the
kernel stays DMA-bound.
"""

import sys

for _p in ("/opt/trn_rl_repo",):
    if _p not in sys.path:
        sys.path.insert(0, _p)

import numpy as np

TAU = 0.1
VTH = 1.5

B, C, H, W, T = 64, 128, 32, 32, 8
NCORES = 8
BS = B // NCORES                      # batches per core
ELEMS = BS * C * H * W * T            # 8,388,608 per core
FREE = 4096                           # tile free dim (cols)
ROWS = ELEMS // FREE                  # 2048
S = FREE // T                         # 512 spatial elems per time slice
P = 128                               # partitions
NTILES = ROWS // P                    # 16

_compiled = None


def _build(reps: int = 1, mode: str = "full"):
    import contextlib

    import concourse.bacc as bacc
    import concourse.mybir as mybir
    import concourse.tile as tile

    nc = bacc.Bacc(
        "TRN2",
        target_bir_lowering=False,
        debug=False,
        num_devices=NCORES,
    )
    f32 = mybir.dt.float32
    x_d = nc.dram_tensor("x", [ROWS, FREE], f32, kind="ExternalInput").ap()
    o_d = nc.dram_tensor("o", [ROWS, FREE], f32, kind="ExternalOutput").ap()

    i8 = mybir.dt.int8
    mult = mybir.AluOpType.mult
    add = mybir.AluOpType.add
    is_gt = mybir.AluOpType.is_gt

    with tile.TileContext(nc) as tc:
        with (
            tc.tile_pool(name="xp", bufs=3) as xp,
            tc.tile_pool(name="mp", bufs=3) as mp,
            tc.tile_pool(name="op", bufs=3) as op_,
            tc.tile_pool(name="zc", bufs=1) as zp,
        ):
            zero = zp.tile([P, S], f32)
            nc.gpsimd.memset(zero[:], 0.0)
            rep_ctx = (
                tc.For_i(0, reps, 1) if reps > 1 else contextlib.nullcontext()
            )
            with rep_ctx:
                _emit_tiles(nc, tc, xp, mp, op_, zero, x_d, o_d, mybir, mode)
    nc.compile()
    return nc


def _emit_tiles(nc, tc, xp, mp, op_, zero, x_d, o_d, mybir, mode="full"):
    f32 = mybir.dt.float32
    i8 = mybir.dt.int8
    mult = mybir.AluOpType.mult
    add = mybir.AluOpType.add
    is_gt = mybir.AluOpType.is_gt
    dma, compute = mode in ("full", "dma"), mode in ("full", "compute")
    if not dma:
        # compute-only: one persistent tile set, recurrence chains on it
        xt = xp.tile([P, FREE], f32)
        nc.gpsimd.memset(xt[:], 0.25)
        mt0 = mp.tile([P, FREE], i8)
        nc.gpsimd.memset(mt0[:], 0)
    for i in range(NTILES):
        if dma:
            xt = xp.tile([P, FREE], f32)
            nc.sync.dma_start(out=xt[:], in_=x_d[i * P : (i + 1) * P, :])
        if compute:
            mt = mp.tile([P, FREE], i8)
            xv = xt[:].rearrange("p (s t) -> p t s", t=T)
            mv = mt[:].rearrange("p (s t) -> p t s", t=T)
            for t in range(T):
                u = xv[:, t]
                if t > 0:
                    up = xv[:, t - 1]
                    # reset where previous step spiked
                    nc.vector.copy_predicated(
                        out=up, mask=mv[:, t - 1], data=zero[:]
                    )
                    # u_t = TAU * u_{t-1} + x_t   (in place into x slice t)
                    nc.vector.scalar_tensor_tensor(
                        out=u, in0=up, scalar=TAU, in1=u, op0=mult, op1=add
                    )
                # o_t = (u_t > VTH) as int8 {0,1}
                nc.vector.tensor_scalar(mv[:, t], u, VTH, None, is_gt)
            # int8 {0,1} -> fp32 spikes, whole tile in one ACT copy
            ot = op_.tile([P, FREE], f32)
            nc.scalar.copy(ot[:], mt[:])
        else:
            ot = xt
        if dma:
            nc.sync.dma_start(out=o_d[i * P : (i + 1) * P, :], in_=ot[:])


def _get_compiled():
    global _compiled
    if _compiled is None:
        _compiled = _build()
    return _compiled


def kernel(x: np.ndarray, _trace: bool = False):
    nc = _get_compiled()
    from concourse.bass_utils import run_bass_kernel_spmd

    x = np.asarray(x, dtype=np.float32)
    in_maps = [
        {"x": np.ascontiguousarray(x[i * BS : (i + 1) * BS]).reshape(ROWS, FREE)}
        for i in range(NCORES)
    ]
    res = run_bass_kernel_spmd(
        nc, in_maps, core_ids=list(range(NCORES)), trace=_trace
    )
    out = np.concatenate(
        [r["o"].reshape(BS, C, H, W, T) for r in res.results], axis=0
    )
    if _trace:
        return out, res
    return out


# revision 11
# speedup vs baseline: 1.7570x; 1.7570x over previous
"""LIF spike (leaky integrate-and-fire) forward kernel for Trainium2.

Recurrence over the innermost time axis T=8 of x[64,128,32,32,8] (fp32):
    u_t = TAU * u_{t-1} * (1 - o_{t-1}) + x_t
    o_t = (u_t > VTH)
Data-parallel over the batch dim: 8 NeuronCores x 8 batches each.

Per-core layout: the 32 MiB shard is viewed as [2048 rows, 4096 cols]
(each row = one (b, c, h-half) slab, cols = spatial*T contiguous). Tiles
of [128, 4096] stream HBM->SBUF; inside SBUF the recurrence walks the
stride-8 time slices in place (the x tile doubles as the membrane-state
buffer), spikes are produced by ScalarE (Sign+Relu) while VectorE does the
reset (copy_predicated) and leak+integrate (scalar_tensor_tensor), so the
kernel stays DMA-bound.
"""

import sys

for _p in ("/opt/trn_rl_repo",):
    if _p not in sys.path:
        sys.path.insert(0, _p)

import numpy as np

TAU = 0.1
VTH = 1.5

B, C, H, W, T = 64, 128, 32, 32, 8
NCORES = 8
BS = B // NCORES                      # batches per core
ELEMS = BS * C * H * W * T            # 8,388,608 per core
FREE = 4096                           # tile free dim (cols)
ROWS = ELEMS // FREE                  # 2048
S = FREE // T                         # 512 spatial elems per time slice
P = 128                               # partitions
NTILES = ROWS // P                    # 16

_compiled = None


def _build(reps: int = 1, mode: str = "full", bufs=(3, 3, 3)):
    import contextlib

    import concourse.bacc as bacc
    import concourse.mybir as mybir
    import concourse.tile as tile

    nc = bacc.Bacc(
        "TRN2",
        target_bir_lowering=False,
        debug=False,
        num_devices=NCORES,
    )
    f32 = mybir.dt.float32
    x_d = nc.dram_tensor("x", [ROWS, FREE], f32, kind="ExternalInput").ap()
    o_d = nc.dram_tensor("o", [ROWS, FREE], f32, kind="ExternalOutput").ap()

    i8 = mybir.dt.int8
    mult = mybir.AluOpType.mult
    add = mybir.AluOpType.add
    is_gt = mybir.AluOpType.is_gt

    with tile.TileContext(nc) as tc:
        with (
            tc.tile_pool(name="xp", bufs=bufs[0]) as xp,
            tc.tile_pool(name="mp", bufs=bufs[1]) as mp,
            tc.tile_pool(name="op", bufs=bufs[2]) as op_,
            tc.tile_pool(name="zc", bufs=1) as zp,
        ):
            zero = zp.tile([P, S], f32)
            nc.gpsimd.memset(zero[:], 0.0)
            rep_ctx = (
                tc.For_i(0, reps, 1) if reps > 1 else contextlib.nullcontext()
            )
            with rep_ctx:
                _emit_tiles(nc, tc, xp, mp, op_, zero, x_d, o_d, mybir, mode)
    nc.compile()
    return nc


def _emit_tiles(nc, tc, xp, mp, op_, zero, x_d, o_d, mybir, mode="full"):
    f32 = mybir.dt.float32
    i8 = mybir.dt.int8
    mult = mybir.AluOpType.mult
    add = mybir.AluOpType.add
    is_gt = mybir.AluOpType.is_gt
    dma, compute = mode in ("full", "dma"), mode in ("full", "compute")
    if not dma:
        # compute-only: one persistent tile set, recurrence chains on it
        xt = xp.tile([P, FREE], f32)
        nc.gpsimd.memset(xt[:], 0.25)
        mt0 = mp.tile([P, FREE], i8)
        nc.gpsimd.memset(mt0[:], 0)
    for i in range(NTILES):
        if dma:
            xt = xp.tile([P, FREE], f32)
            nc.sync.dma_start(out=xt[:], in_=x_d[i * P : (i + 1) * P, :])
        if compute:
            mt = mp.tile([P, FREE], i8)
            xv = xt[:].rearrange("p (s t) -> p t s", t=T)
            mv = mt[:].rearrange("p (s t) -> p t s", t=T)
            for t in range(T):
                u = xv[:, t]
                if t > 0:
                    up = xv[:, t - 1]
                    # reset where previous step spiked
                    nc.vector.copy_predicated(
                        out=up, mask=mv[:, t - 1], data=zero[:]
                    )
                    # u_t = TAU * u_{t-1} + x_t   (in place into x slice t)
                    nc.vector.scalar_tensor_tensor(
                        out=u, in0=up, scalar=TAU, in1=u, op0=mult, op1=add
                    )
                # o_t = (u_t > VTH) as int8 {0,1}
                nc.vector.tensor_scalar(mv[:, t], u, VTH, None, is_gt)
            # int8 {0,1} -> fp32 spikes, whole tile in one ACT copy
            ot = op_.tile([P, FREE], f32)
            nc.scalar.copy(ot[:], mt[:])
        else:
            ot = xt
        if dma:
            nc.sync.dma_start(out=o_d[i * P : (i + 1) * P, :], in_=ot[:])


def _get_compiled():
    global _compiled
    if _compiled is None:
        _compiled = _build()
    return _compiled


def kernel(x: np.ndarray, _trace: bool = False):
    nc = _get_compiled()
    from concourse.bass_utils import run_bass_kernel_spmd

    x = np.asarray(x, dtype=np.float32)
    in_maps = [
        {"x": np.ascontiguousarray(x[i * BS : (i + 1) * BS]).reshape(ROWS, FREE)}
        for i in range(NCORES)
    ]
    res = run_bass_kernel_spmd(
        nc, in_maps, core_ids=list(range(NCORES)), trace=_trace
    )
    out = np.concatenate(
        [r["o"].reshape(BS, C, H, W, T) for r in res.results], axis=0
    )
    if _trace:
        return out, res
    return out


# revision 12
# speedup vs baseline: 7.0114x; 3.9905x over previous
"""LIF spike (leaky integrate-and-fire) forward kernel for Trainium2.

Recurrence over the time axis T=8 of x[64,128,32,32,8] (fp32):
    u_t = TAU * u_{t-1} * (1 - o_{t-1}) + x_t
    o_t = (u_t > VTH)
Data-parallel over the batch dim: 8 NeuronCores x 8 batches each.

Layout: the host transposes each core's shard to time-major [T, spatial]
so that every time-step slice is a contiguous [128, FD] tile (unit-stride
APs for every engine op, contiguous >=1MiB DMAs). Per step the work is:
    o_t  = (u_t > VTH)                 DVE tensor_scalar is_gt -> fp32 out
    w_t  = TAU - TAU*o_t               ScalarE activation Copy(scale,bias),
                                       written in place over o_t after its
                                       store DMA has read it
    u_'  = u_t * w_t                   DVE tensor_tensor mult (in place)
    u_t1 = u_' + x_t1                  DVE tensor_tensor add (in place on
                                       the freshly loaded x tile)
The x tile doubles as the membrane-state buffer, the o tile doubles as the
w buffer, so SBUF holds just two fp32 pools.
"""

import sys

for _p in ("/opt/trn_rl_repo",):
    if _p not in sys.path:
        sys.path.insert(0, _p)

import numpy as np

TAU = 0.1
VTH = 1.5

B, C, H, W, T = 64, 128, 32, 32, 8
NCORES = 8
BS = B // NCORES                      # batches per core
SPAT = BS * C * H * W                 # spatial elems per core per step: 1,048,576
P = 128                               # partitions
FD = 2048                             # free dim per tile
NCH = SPAT // (P * FD)                # spatial chunks per step: 4
ROWS = T * NCH * P                    # dram rows (t-major): 4096
ELEMS = SPAT * T

_compiled = None


def _build(reps: int = 1, mode: str = "full", bufs=(10, 10), fd=FD):
    import contextlib

    import concourse.bacc as bacc
    import concourse.mybir as mybir
    import concourse.tile as tile

    nch = SPAT // (P * fd)
    nc = bacc.Bacc(
        "TRN2",
        target_bir_lowering=False,
        debug=False,
        num_devices=NCORES,
    )
    f32 = mybir.dt.float32
    x_d = nc.dram_tensor("x", [T * nch * P, fd], f32, kind="ExternalInput").ap()
    o_d = nc.dram_tensor("o", [T * nch * P, fd], f32, kind="ExternalOutput").ap()

    with tile.TileContext(nc) as tc:
        with (
            tc.tile_pool(name="xp", bufs=bufs[0]) as xp,
            tc.tile_pool(name="op", bufs=bufs[1]) as op_,
        ):
            rep_ctx = (
                tc.For_i(0, reps, 1) if reps > 1 else contextlib.nullcontext()
            )
            with rep_ctx:
                _emit(nc, xp, op_, x_d, o_d, mybir, mode, fd, nch)
    nc.compile()
    return nc


def _emit(nc, xp, op_, x_d, o_d, mybir, mode, fd, nch):
    f32 = mybir.dt.float32
    mult = mybir.AluOpType.mult
    add = mybir.AluOpType.add
    is_gt = mybir.AluOpType.is_gt
    copy_f = mybir.ActivationFunctionType.Copy
    dma, compute = mode in ("full", "dma"), mode in ("full", "compute")

    u = [None] * nch       # tile holding u_t per chunk
    o_prev = [None] * nch  # tile holding o_{t-1} per chunk
    for t in range(T):
        for c in range(nch):
            r0 = (t * nch + c) * P
            xt = xp.tile([P, fd], f32)
            if dma:
                nc.sync.dma_start(out=xt[:], in_=x_d[r0 : r0 + P, :])
            elif t == 0:
                nc.gpsimd.memset(xt[:], 0.25)
            if compute:
                if t == 0:
                    u[c] = xt
                else:
                    w = o_prev[c]
                    # w <- TAU - TAU*o  (in place over o after its store)
                    nc.scalar.activation(
                        w[:], w[:], copy_f, bias=TAU, scale=-TAU
                    )
                    # u_masked = u_{t-1} * w   (in place)
                    nc.vector.tensor_tensor(
                        out=u[c][:], in0=u[c][:], in1=w[:], op=mult
                    )
                    # u_t = u_masked + x_t    (in place on x tile)
                    nc.vector.tensor_tensor(
                        out=xt[:], in0=u[c][:], in1=xt[:], op=add
                    )
                    u[c] = xt
                ot = op_.tile([P, fd], f32)
                nc.vector.tensor_scalar(ot[:], u[c][:], VTH, None, is_gt)
                o_prev[c] = ot
            else:
                ot = xt
            if dma:
                nc.sync.dma_start(out=o_d[r0 : r0 + P, :], in_=ot[:])


def _get_compiled():
    global _compiled
    if _compiled is None:
        _compiled = _build()
    return _compiled


def _shard_tmajor(x: np.ndarray, i: int) -> np.ndarray:
    """Core i's shard, time-major: [T*NCH*P, FD], row-major over (t, spatial)."""
    xs = x[i * BS : (i + 1) * BS]                   # [BS,C,H,W,T]
    xt = np.moveaxis(xs.reshape(SPAT, T), -1, 0)    # [T, SPAT]
    return np.ascontiguousarray(xt).reshape(ROWS, FD)


def kernel(x: np.ndarray):
    nc = _get_compiled()
    from concourse.bass_utils import run_bass_kernel_spmd

    x = np.asarray(x, dtype=np.float32)
    in_maps = [{"x": _shard_tmajor(x, i)} for i in range(NCORES)]
    res = run_bass_kernel_spmd(nc, in_maps, core_ids=list(range(NCORES)))
    outs = []
    for r in res.results:
        ot = r["o"].reshape(T, SPAT)                # time-major back to T-last
        outs.append(np.moveaxis(ot, 0, -1).reshape(BS, C, H, W, T))
    return np.ascontiguousarray(np.concatenate(outs, axis=0))
